# revision 1
# baseline (speedup 1.0000x reference)
"""Multi-head attention (QKV proj + per-head RMSNorm + softmax attention +
output proj) for Trainium2, distributed over 8 NeuronCores.

Sharding: batch (2) x head-groups (4 heads per core).  Each core computes, for
its batch element and its 4 heads: Q^T/K^T projections (transposed layout
[d, T], from a host-pretransposed X^T), per-head RMSNorm folded in as a
broadcast-matmul column scale, S^T = K^T.T @ Q^T scores in [key, query]
layout (so softmax normalizers fall out of a fused [V|1] matmul and no P
transposes are ever needed), exp with no max subtraction (RMSNorm bounds
|score/8| <= 8 via Cauchy-Schwarz), O^T accumulation, normalization, and a
partial output projection Out^T = Wo_slice.T @ O^T.  The host sums the 4
partial outputs per batch and transposes back.

GEMM hot path runs in bf16 operands with fp32 PSUM accumulation (matmul
weights zero-padded to 128 contraction rows: 64-row weights run ~2x slower).
The RMSNorm statistics path stays fp32/float32r.  exp/ln run on ScalarE
(rsqrt via exp(-0.5*ln(x)) - far cheaper than DVE reciprocal).  The
attention phase is ScalarE-bound (exp over T^2 scores); projections/output
phases are TensorE-bound.
"""

import os
import sys

for _p in ("/opt/trn_rl_repo",):
    if _p not in sys.path:
        sys.path.insert(0, _p)

import numpy as np

B = 2
T = 2048
D = 1024
H = 16
HD = 64
HPC = 4          # heads per core
N_CORES = 8
EPS = 1e-5

_COMPILED = None
LAST_EXEC_NS = None
HOT_BF16 = os.environ.get("ATTN_HOT", "bf16") == "bf16"


def _install_ntff_shim():
    """antenv.axon_hooks is missing in this image; provide it so that
    BASS_TRACE=1 profiling works (mirrors trn_boot's ctypes hook)."""
    import contextlib
    import ctypes
    import types

    if "antenv.axon_hooks" in sys.modules:
        return
    so_path = "/opt/axon/libaxon_pjrt.so"
    if not os.path.exists(so_path):
        return
    lib = ctypes.CDLL(so_path)
    if not hasattr(lib, "axon_start_nrt_profile"):
        return
    lib.axon_start_nrt_profile.argtypes = [ctypes.POINTER(ctypes.c_int64), ctypes.c_size_t]
    lib.axon_start_nrt_profile.restype = ctypes.c_int64
    lib.axon_stop_nrt_profile.argtypes = [ctypes.c_char_p]
    lib.axon_stop_nrt_profile.restype = ctypes.c_int64

    @contextlib.contextmanager
    def _hook(output_dir, device_ids):
        import jax

        jax.devices()
        if device_ids:
            ids = (ctypes.c_int64 * len(device_ids))(*device_ids)
            rc = lib.axon_start_nrt_profile(ids, len(device_ids))
        else:
            rc = lib.axon_start_nrt_profile(None, 0)
        if rc != 0:
            raise RuntimeError(f"axon_start_nrt_profile rc={rc}")
        try:
            yield
        finally:
            n = lib.axon_stop_nrt_profile(str(output_dir).encode())
            print(f"profile: {n} file(s) written to {output_dir}", file=sys.stderr)

    mod = types.ModuleType("antenv.axon_hooks")
    mod._hook = _hook
    mod.get_axon_ntff_profile_hook = lambda: mod._hook
    mod.set_axon_ntff_profile_hook = lambda h: setattr(mod, "_hook", h)
    sys.modules["antenv.axon_hooks"] = mod
    try:
        import antenv

        antenv.axon_hooks = mod
    except ImportError:
        pass


def _build():
    import concourse.bass as bass
    import concourse.tile as tile
    from concourse import bacc, mybir

    F32 = mybir.dt.float32
    F32R = mybir.dt.float32r
    BF16 = mybir.dt.bfloat16
    HOT = BF16 if HOT_BF16 else F32R
    Exp = mybir.ActivationFunctionType.Exp
    Log = mybir.ActivationFunctionType.Ln if hasattr(
        mybir.ActivationFunctionType, "Ln") else mybir.ActivationFunctionType.Log

    TT = T // 128            # 16 t-tiles
    CT = D // 128            # 8 contraction tiles over model dim
    QH = T // 1024           # 2 query halves
    NPAIR = HPC // 2         # 2 head pairs per core

    nc = bacc.Bacc("TRN2", target_bir_lowering=False, debug=False, num_devices=N_CORES)

    HIN = BF16 if HOT_BF16 else F32
    xbT = nc.dram_tensor("xbT", (D, T), HIN, kind="ExternalInput").ap()
    wq_s = nc.dram_tensor("wq_s", (D, HPC * HD), HIN, kind="ExternalInput").ap()
    wk_s = nc.dram_tensor("wk_s", (D, HPC * HD), HIN, kind="ExternalInput").ap()
    wv_s = nc.dram_tensor("wv_s", (D, HPC * HD), HIN, kind="ExternalInput").ap()
    wo_s = nc.dram_tensor("wo_s", (HPC * HD, D), HIN, kind="ExternalInput").ap()
    ident_d = nc.dram_tensor("ident", (128, 128), HIN, kind="ExternalInput").ap()
    bd2_d = nc.dram_tensor("bd2", (128, 2), F32, kind="ExternalInput").ap()
    wqc_d = nc.dram_tensor("wqc", (128, 1), F32, kind="ExternalInput").ap()
    wkc_d = nc.dram_tensor("wkc", (128, 1), F32, kind="ExternalInput").ap()
    sel_d = [nc.dram_tensor(f"sel{p}", (128, 128), F32, kind="ExternalInput").ap()
             for p in range(NPAIR)]
    if not HOT_BF16:
        onec_d = nc.dram_tensor("onec", (128, 1), HIN, kind="ExternalInput").ap()
    outT = nc.dram_tensor("outT", (D, T), HIN, kind="ExternalOutput").ap()

    with tile.TileContext(nc) as tc:
        from contextlib import ExitStack

        with ExitStack() as top:
            # ---- persistent pools -------------------------------------------------
            consts = top.enter_context(tc.tile_pool(name="consts", bufs=1))
            qkpool = top.enter_context(tc.tile_pool(name="qk", bufs=1))
            vppool = top.enter_context(tc.tile_pool(name="vp", bufs=1))
            drp = top.enter_context(tc.tile_pool(name="drs", bufs=1, space="DRAM"))


            ident = consts.tile([128, 128], HOT, tag="ident")
            nc.sync.dma_start(out=ident[:], in_=ident_d.bitcast(HOT))
            epsc = consts.tile([128, 1], F32, tag="epsc")
            nc.vector.memset(epsc[:], EPS)
            bd2 = consts.tile([128, 2], F32R, tag="bd2")
            nc.sync.dma_start(out=bd2[:], in_=bd2_d.bitcast(F32R))
            wqc = consts.tile([128, 1], F32, tag="wqc")
            nc.sync.dma_start(out=wqc[:], in_=wqc_d)
            wkc = consts.tile([128, 1], F32, tag="wkc")
            nc.sync.dma_start(out=wkc[:], in_=wkc_d)
            sel = []
            for p in range(NPAIR):
                s = consts.tile([128, 128], F32R, tag=f"sel{p}", name=f"sel{p}")
                nc.sync.dma_start(out=s[:], in_=sel_d[p].bitcast(F32R))
                sel.append(s)

            # persistent data tiles
            # per-head tiles, zero-padded to full 128 contraction rows:
            # 64-row matmul weights run ~2x slower than 128-row (no FWL /
            # no LDW overlap), so pad with zeros and contract over 128.
            qhat = [qkpool.tile([128, T], HOT, tag=f"qh{h}", name=f"qhat{h}")
                    for h in range(HPC)]
            khat = [qkpool.tile([128, T], HOT, tag=f"kh{h}", name=f"khat{h}")
                    for h in range(HPC)]
            for h in range(HPC):
                nc.vector.memset(qhat[h][:], 0.0)
                nc.gpsimd.memset(khat[h][:], 0.0)
            # V staging: [128 keys, TT, 2, 65]; [:,tt,h,:] = [V_h|1]
            vp = [vppool.tile([128, TT, 2, 65], HOT, tag=f"vs{p}", name=f"vst{p}")
                  for p in range(NPAIR)]
            for p in range(NPAIR):
                nc.vector.memset(vp[p][:, :, :, 64:65], 1.0)

            # =============== Phase 0+1: X^T, projections, RMS norm ================
            with ExitStack() as p01:
                ps_big = p01.enter_context(
                    tc.tile_pool(name="psbig", bufs=3, space="PSUM"))
                ps_sml = p01.enter_context(
                    tc.tile_pool(name="pssml", bufs=2, space="PSUM"))
                xtp = p01.enter_context(tc.tile_pool(name="xT", bufs=CT))
                wpool = p01.enter_context(tc.tile_pool(name="w", bufs=CT))
                qtp = p01.enter_context(tc.tile_pool(name="qt", bufs=4))
                vtp = p01.enter_context(tc.tile_pool(name="vt", bufs=2))
                q2p = p01.enter_context(tc.tile_pool(name="q2", bufs=2))
                nsml = p01.enter_context(tc.tile_pool(name="nsml", bufs=1))

                # ---- X^T: loaded directly (host provides transposed x) ----
                xT = [xtp.tile([128, T], HOT, tag="xT", name=f"xT{c}")
                      for c in range(CT)]
                for cb in range(CT):
                    nc.sync.dma_start(out=xT[cb][:],
                                      in_=xbT[cb * 128:(cb + 1) * 128, :])

                # ---- projections ----
                def project(w_dram, name):
                    """returns per-pair psum eviction targets via callback loop"""
                    wts = []
                    for ct in range(CT):
                        wt = wpool.tile([128, HPC * HD], HOT, tag=f"w{name}",
                                        name=f"w{name}{ct}")
                        nc.sync.dma_start(
                            out=wt[:], in_=w_dram[ct * 128:(ct + 1) * 128, :].bitcast(HOT))
                        wts.append(wt)
                    out_ps = {}
                    for pair in range(NPAIR):
                        for qh in range(QH):
                            pj = ps_big.tile([128, 1024], F32, tag="big")
                            for ct in range(CT):
                                for qq in range(2):
                                    nc.tensor.matmul(
                                        pj[:, qq * 512:(qq + 1) * 512],
                                        wts[ct][:, pair * 128:(pair + 1) * 128],
                                        xT[ct][:, qh * 1024 + qq * 512:
                                               qh * 1024 + (qq + 1) * 512],
                                        start=(ct == 0), stop=(ct == CT - 1))
                            yield pair, qh, pj

                def proj_stage(w_dram, name):
                    """projection + raw evict + sumsq; returns (qt tiles, ms tiles)"""
                    qt_cur = {}
                    ms_sb = {}
                    for pair, qh, pj in project(w_dram, name):
                        if qh == 0:
                            qt_cur[pair] = qtp.tile([128, T], F32, tag="qt",
                                                     name=f"qt{name}{pair}")
                            ms_sb[pair] = nsml.tile([2, T], F32, tag=f"ms{name}{pair}",
                                                    name=f"ms{name}{pair}")
                        qt_sb = qt_cur[pair]
                        sl = slice(qh * 1024, (qh + 1) * 1024)
                        nc.scalar.copy(qt_sb[:, sl], pj[:])
                        q2 = q2p.tile([128, 1024], F32R, tag="q2")
                        with nc.allow_low_precision(reason="fp32r rounding"):
                            nc.vector.tensor_mul(q2[:], qt_sb[:, sl], qt_sb[:, sl])
                        for qq in range(2):
                            ss = ps_sml.tile([2, 512], F32, tag="sml")
                            nc.tensor.matmul(ss[:], bd2[:], q2[:, qq * 512:(qq + 1) * 512],
                                             start=True, stop=True)
                            nc.vector.tensor_copy(
                                out=ms_sb[pair][:, qh * 1024 + qq * 512:
                                                qh * 1024 + (qq + 1) * 512],
                                in_=ss[:])
                    return qt_cur, ms_sb

                def norm_stage(qt_cur, ms_sb, wcol, dest, name):
                    # rstd = (ms/64+eps)^-1/2 = exp(-0.5*ln(ms/64+eps))
                    rstds = {}
                    for pair in range(NPAIR):
                        nc.scalar.activation(ms_sb[pair][:], ms_sb[pair][:], Log,
                                             scale=1.0 / HD, bias=epsc[0:2, :])
                    for pair in range(NPAIR):
                        rstd = nsml.tile([2, T], F32, tag=f"rstd{name}{pair}",
                                         name=f"rstd{name}{pair}")
                        nc.scalar.activation(rstd[:], ms_sb[pair][:], Exp, scale=-0.5)
                        rstd_d = drp.tile([2, T], F32, tag=f"rstdd{name}{pair}",
                                          name=f"rstdd{name}{pair}")
                        nc.sync.dma_start(out=rstd_d[:], in_=rstd[:])
                        rstds[pair] = rstd_d
                    mult = mybir.AluOpType.mult
                    for pair in range(NPAIR):
                        for qh in range(QH):
                            sl = slice(qh * 1024, (qh + 1) * 1024)
                            # partition-broadcast the [1, t] rstd rows via DMA
                            rwsb = q2p.tile([128, 1024], F32, tag="rwsb", bufs=2)
                            for i in range(2):
                                row = rstds[pair][i:i + 1, sl]
                                brd = bass.AP(tensor=row.tensor, offset=row.offset,
                                              ap=[[0, 64]] + list(row.ap[1:]))
                                nc.sync.dma_start(out=rwsb[64 * i:64 * i + 64, :],
                                                  in_=brd)
                            with nc.allow_low_precision(reason="fp32r rounding"):
                                for i in range(2):
                                    rows = slice(64 * i, 64 * i + 64)
                                    nc.vector.scalar_tensor_tensor(
                                        out=dest[pair * 2 + i][rows, sl],
                                        in0=qt_cur[pair][rows, sl],
                                        scalar=wcol[rows, :],
                                        in1=rwsb[rows, :],
                                        op0=mult, op1=mult)

                qt_q, ms_q = proj_stage(wq_s, "q")
                qt_k, ms_k = proj_stage(wk_s, "k")

                norm_stage(qt_q, ms_q, wqc, qhat, "q")
                norm_stage(qt_k, ms_k, wkc, khat, "k")

                # ---- V: project to V^T then transpose into [V|1] tiles ----
                for pair, qh, pj in project(wv_s, "v"):
                    if qh == 0:
                        vt_sb = vtp.tile([128, T], HOT, tag="vt")
                        vt_cur = vt_sb
                    else:
                        vt_sb = vt_cur
                    nc.scalar.copy(vt_sb[:, qh * 1024:(qh + 1) * 1024], pj[:])
                    if qh == QH - 1:
                        for tt in range(TT):
                            if tt % 4 == 0:
                                vb_ps = ps_big.tile([128, 512], HOT, tag="big",
                                                    name="vb_ps")
                            j = tt % 4
                            nc.tensor.transpose(
                                vb_ps[:, j * 128:(j + 1) * 128],
                                vt_sb[:, tt * 128:(tt + 1) * 128], ident[:])
                            bl = vb_ps[:, j * 128:(j + 1) * 128]
                            with nc.allow_low_precision(reason="fp32r rounding"):
                                nc.scalar.copy(
                                    vp[pair][:, tt, :, 0:64],
                                    bl.rearrange("p (h d) -> p h d", h=2))


            # scheduler fence: nothing from P2/P3 may be hoisted before P0/P1
            tc.no_sync_barrier()

            # =============== Phase 2+3: attention + output projection =============
            with ExitStack() as p23:
                ppool = p23.enter_context(tc.tile_pool(name="p", bufs=4))
                dntp = p23.enter_context(tc.tile_pool(name="dnt", bufs=3))
                ps_sbig = p23.enter_context(
                    tc.tile_pool(name="pssbig", bufs=3, space="PSUM"))
                ps_o = p23.enter_context(
                    tc.tile_pool(name="pso", bufs=1, space="PSUM"))
                dnp = p23.enter_context(tc.tile_pool(name="dn", bufs=1))
                ohpool = p23.enter_context(tc.tile_pool(name="ohp", bufs=1))
                outsbp = p23.enter_context(tc.tile_pool(name="outsb", bufs=3))
                wop = p23.enter_context(tc.tile_pool(name="wo", bufs=2))

                dn_all = [dnp.tile([2, T], F32, tag=f"dn{p}", name=f"dn{p}")
                          for p in range(NPAIR)]
                wo_sb = []
                for cp in range(NPAIR):
                    wt = wop.tile([128, D], HOT, tag="wo", name=f"wo{cp}")
                    nc.sync.dma_start(out=wt[:],
                                      in_=wo_s[cp * 128:(cp + 1) * 128, :].bitcast(HOT))
                    wo_sb.append(wt)
                ohp = [ohpool.tile([128, T], F32, tag=f"ohp{p}", name=f"ohp{p}")
                       for p in range(NPAIR)]
                ohr = [ohpool.tile([128, T], HOT, tag=f"ohr{p}", name=f"ohr{p}")
                       for p in range(NPAIR)]

                for h in range(HPC):
                    pair, i = h // 2, h % 2
                    Ks = khat[h]
                    Qs = qhat[h]
                    for qh in range(QH):
                        o_ps = ps_o.tile([128, 1024], F32, tag="o")
                        orows = slice(0, 65)
                        vcol = i
                        for kt in range(TT):
                            s_ps = ps_sbig.tile([128, 1024], F32, tag="sbig")
                            for qq in range(2):
                                nc.tensor.matmul(
                                    s_ps[:, qq * 512:(qq + 1) * 512],
                                    Ks[:, kt * 128:(kt + 1) * 128],
                                    Qs[:, qh * 1024 + qq * 512:qh * 1024 + (qq + 1) * 512],
                                    start=True, stop=True)
                            p_sb = ppool.tile([128, 1024], HOT, tag="p")
                            nc.scalar.activation(p_sb[:], s_ps[:], Exp, scale=0.125)
                            for qq in range(2):
                                nc.tensor.matmul(
                                    o_ps[orows, qq * 512:(qq + 1) * 512],
                                    vp[pair][:, kt, vcol, :],
                                    p_sb[:, qq * 512:(qq + 1) * 512],
                                    start=(kt == 0), stop=(kt == TT - 1))
                        # evict raw O^T rows + denominator row; the final
                        # head's evictions go to ACT (idle once exps are done)
                        sl = slice(qh * 1024, (qh + 1) * 1024)
                        ev = nc.scalar.copy if h == HPC - 1 else (
                            lambda o_, i_: nc.vector.tensor_copy(out=o_, in_=i_))
                        dnt = dntp.tile([65, 1024], F32, tag="dnt")
                        ev(dnt[64:65, :], o_ps[64:65, :])
                        nc.sync.dma_start(out=dn_all[pair][i:i + 1, sl],
                                          in_=dnt[64:65, :])
                        if i == 0:
                            ev(ohp[pair][0:64, sl], o_ps[0:64, :])
                        else:
                            # cross-partition move: tmp rows 0:64, DMA remap
                            ev(dnt[0:64, :], o_ps[0:64, :])
                            nc.sync.dma_start(out=ohp[pair][64:128, sl],
                                              in_=dnt[0:64, :])

                    if pair == 0 and i == 1 and qh == QH - 1:
                        # pair0 denominators ready mid-attention: reciprocal on
                        # the idle DVE so tail rb matmuls start immediately
                        dnr0 = dnp.tile([128, T], F32R, tag="dnr0", name="dnr0")
                        nc.gpsimd.memset(dnr0[:, :].bitcast(F32), 0.0)
                        with nc.allow_low_precision(reason="fp32r rounding"):
                            nc.vector.reciprocal(dnr0[0:2, :], dn_all[0][:])

                for pair in range(NPAIR):
                    if pair == 0:
                        dnr = dnr0
                    else:
                        # 1/d = exp(-ln(d)) on ACT (idle once exps finish)
                        nc.scalar.activation(dn_all[pair][:], dn_all[pair][:],
                                             Log, scale=1.0)
                        dnr = dnp.tile([128, T], F32R, tag=f"dnr{pair}",
                                       name=f"dnr{pair}")
                        nc.gpsimd.memset(dnr[:, :].bitcast(F32), 0.0)
                        with nc.allow_low_precision(reason="fp32r rounding"):
                            nc.scalar.activation(dnr[0:2, :], dn_all[pair][:],
                                                 Exp, scale=-1.0)
                    for qh2 in range(QH):
                        sl2 = slice(qh2 * 1024, (qh2 + 1) * 1024)
                        rb = ps_sbig.tile([128, 1024], F32, tag="sbig")
                        for qq in range(2):
                            nc.tensor.matmul(
                                rb[:, qq * 512:(qq + 1) * 512], sel[pair][:],
                                dnr[:, qh2 * 1024 + qq * 512:
                                    qh2 * 1024 + (qq + 1) * 512],
                                start=True, stop=True)
                        with nc.allow_low_precision(reason="fp32r rounding"):
                            nc.vector.tensor_mul(ohr[pair][:, sl2],
                                                 ohp[pair][:, sl2], rb[:])

                # ---- output projection: Out^T = wo_s.T @ Ohat^T ----
                # [128, 1024] psum tiles, 2 same-weight matmuls per LDW
                for et in range(D // 128):
                    osb = outsbp.tile([128, T], HOT, tag="outsb")
                    for th in range(T // 1024):
                        ops = ps_sbig.tile([128, 1024], F32, tag="sbig")
                        for cp in range(NPAIR):
                            for qq in range(2):
                                nc.tensor.matmul(
                                    ops[:, qq * 512:(qq + 1) * 512],
                                    wo_sb[cp][:, et * 128:(et + 1) * 128],
                                    ohr[cp][:, th * 1024 + qq * 512:
                                            th * 1024 + (qq + 1) * 512],
                                    start=(cp == 0), stop=(cp == NPAIR - 1))
                        with nc.allow_low_precision(reason="bf16 partial output"):
                            if th % 2 == 0:
                                nc.vector.tensor_copy(
                                    out=osb[:, th * 1024:(th + 1) * 1024], in_=ops[:])
                            else:
                                nc.scalar.copy(osb[:, th * 1024:(th + 1) * 1024],
                                               ops[:])
                        (nc.scalar if th % 2 == 0 else nc.sync).dma_start(
                            out=outT[et * 128:(et + 1) * 128,
                                     th * 1024:(th + 1) * 1024],
                            in_=osb[:, th * 1024:(th + 1) * 1024])

    nc.compile()
    return nc


def _get_compiled():
    global _COMPILED
    if _COMPILED is None:
        _COMPILED = _build()
    return _COMPILED


def _make_consts(q_norm_w, k_norm_w):
    ident = np.eye(128, dtype=np.float32)
    bd2 = np.zeros((128, 2), np.float32)
    bd2[0:64, 0] = 1.0
    bd2[64:128, 1] = 1.0
    wqb = np.zeros((128, 128), np.float32)
    wqb[0, 0:64] = q_norm_w
    wqb[1, 64:128] = q_norm_w
    wkb = np.zeros((128, 128), np.float32)
    wkb[0, 0:64] = k_norm_w
    wkb[1, 64:128] = k_norm_w
    sels = []
    for p in range(2):
        s = np.zeros((128, 128), np.float32)
        s[0, 0:64] = 1.0
        s[1, 64:128] = 1.0
        sels.append(s)
    onec = np.ones((128, 1), np.float32)
    wqc = np.concatenate([q_norm_w, q_norm_w]).reshape(128, 1).astype(np.float32)
    wkc = np.concatenate([k_norm_w, k_norm_w]).reshape(128, 1).astype(np.float32)
    sels = []
    for p in range(2):
        s = np.zeros((128, 128), np.float32)
        s[0, 0:64] = 1.0
        s[1, 64:128] = 1.0
        sels.append(s)
    return ident, bd2, onec, wqc, wkc, sels


def kernel(x, wq, wk, wv, wo, q_norm_w, k_norm_w):
    from concourse.bass_utils import run_bass_kernel_spmd

    global LAST_EXEC_NS
    if os.environ.get("BASS_TRACE"):
        _install_ntff_shim()

    x = np.asarray(x, dtype=np.float32)
    wq = np.asarray(wq, dtype=np.float32)
    wk = np.asarray(wk, dtype=np.float32)
    wv = np.asarray(wv, dtype=np.float32)
    wo = np.asarray(wo, dtype=np.float32)
    q_norm_w = np.asarray(q_norm_w, dtype=np.float32)
    k_norm_w = np.asarray(k_norm_w, dtype=np.float32)

    nc = _get_compiled()
    ident, bd2, onec, wqc, wkc, sels = _make_consts(q_norm_w, k_norm_w)
    if HOT_BF16:
        import ml_dtypes
        cast = lambda a: a.astype(ml_dtypes.bfloat16)
    else:
        cast = lambda a: a

    in_maps = []
    for c in range(N_CORES):
        b = c // 4
        hs = HPC * (c % 4)
        # head split in reference is strided: head h uses columns d*H + h
        perm = ((hs + np.arange(HPC))[:, None] + H * np.arange(HD)[None, :]).reshape(-1)
        in_maps.append({
            "xbT": cast(np.ascontiguousarray(x[b].T)),
            "wq_s": cast(np.ascontiguousarray(wq[:, perm])),
            "wk_s": cast(np.ascontiguousarray(wk[:, perm])),
            "wv_s": cast(np.ascontiguousarray(wv[:, perm])),
            "wo_s": cast(np.ascontiguousarray(wo[hs * HD:(hs + HPC) * HD, :])),
            "ident": cast(ident), "bd2": bd2,
            "wqc": wqc, "wkc": wkc, "sel0": sels[0], "sel1": sels[1],
            **({} if HOT_BF16 else {"onec": cast(onec)}),
        })

    res = run_bass_kernel_spmd(nc, in_maps, core_ids=list(range(N_CORES)),
                               trace=bool(os.environ.get("BASS_TRACE")),
                               tmpdir=os.environ.get("BASS_TRACE_DIR"))
    LAST_EXEC_NS = res.exec_time_ns

    out = np.empty((B, T, D), dtype=np.float32)
    for b in range(B):
        acc = res.results[4 * b]["outT"].astype(np.float32)
        for c in range(4 * b + 1, 4 * b + 4):
            acc = acc + res.results[c]["outT"].astype(np.float32)
        out[b] = acc.T
    return out



# revision 10
# speedup vs baseline: 1.1312x; 1.1312x over previous
"""Multi-head attention (QKV proj + per-head RMSNorm + softmax attention +
output proj) for Trainium2, distributed over 8 NeuronCores.

Sharding: batch (2) x head-groups (4 heads per core).  Per core, for its batch
element and 4 heads (2 pairs):

- All matmuls run in bf16 with fp32 PSUM accumulation (fp8 was measured to
  cost ~3.5% output rms: softmax output is itself a weighted mean, so
  per-element P/V/Qhat quantization error does NOT average down over keys).
- Projections go through [128,512] PSUM tiles, evicted to bf16 SBUF staging.
- Per-head RMSNorm: sumsq via a [128,2] ones-block matmul; rstd is computed
  on GPSIMD (quake rsqrt seed + 2 sign-cancelling Newton steps) on a
  DMA-packed [128,32] tile so the DVE queue never blocks on the DMA
  round-trip; the result is DMA partition-broadcast and folded into Q/K by a
  DVE scalar_tensor_tensor multiply (all-bf16, 4x DVE mode).
- Attention in S^T = [key, query] layout: scores contract over 128 rows
  (64 real + 64 zero-padded, free since PE time only depends on free size),
  exp runs on ACT (ScalarE) from PSUM f32 to bf16 P tiles - ACT does only
  exp + a few same-table copies, no activation-table thrashing.  O^T
  accumulates with [V|1] weights so softmax denominators fall out of the
  same matmul (no extra pass).
- Denominator reciprocals: DMA-pack the 4 per-head rows into [128,32], DVE
  reciprocal_approx_accurate, DMA-broadcast back; O is normalized by a
  4x-mode DVE multiply.  The output projection runs per token-half so the
  first half overlaps the second half's attention.
- Loop order is query-half outer, head inner; pair-1 projections are emitted
  as fine-grained chunks interleaved under heads 0-1 of the first query half
  so TensorE (the global bottleneck at ~170us busy) never idles.
"""

import os
import sys

for _p in ("/opt/trn_rl_repo",):
    if _p not in sys.path:
        sys.path.insert(0, _p)

import numpy as np

B = 2
T = 2048
D = 1024
H = 16
HD = 64
HPC = 4          # heads per core
NPAIR = 2
N_CORES = 8
EPS = 1e-5
TT = T // 128    # 16 key tiles
CT = D // 128    # 8 contraction tiles
QH = T // 1024   # 2 query halves

_COMPILED = None
LAST_EXEC_NS = None


def _install_ntff_shim():
    """antenv.axon_hooks is missing in this image; provide it so that
    BASS_TRACE=1 profiling works (mirrors trn_boot's ctypes hook)."""
    import contextlib
    import ctypes
    import types

    if "antenv.axon_hooks" in sys.modules:
        return
    so_path = "/opt/axon/libaxon_pjrt.so"
    if not os.path.exists(so_path):
        return
    lib = ctypes.CDLL(so_path)
    if not hasattr(lib, "axon_start_nrt_profile"):
        return
    lib.axon_start_nrt_profile.argtypes = [ctypes.POINTER(ctypes.c_int64), ctypes.c_size_t]
    lib.axon_start_nrt_profile.restype = ctypes.c_int64
    lib.axon_stop_nrt_profile.argtypes = [ctypes.c_char_p]
    lib.axon_stop_nrt_profile.restype = ctypes.c_int64

    @contextlib.contextmanager
    def _hook(output_dir, device_ids):
        import jax

        jax.devices()
        if device_ids:
            ids = (ctypes.c_int64 * len(device_ids))(*device_ids)
            rc = lib.axon_start_nrt_profile(ids, len(device_ids))
        else:
            rc = lib.axon_start_nrt_profile(None, 0)
        if rc != 0:
            raise RuntimeError(f"axon_start_nrt_profile rc={rc}")
        try:
            yield
        finally:
            n = lib.axon_stop_nrt_profile(str(output_dir).encode())
            print(f"profile: {n} file(s) written to {output_dir}", file=sys.stderr)

    mod = types.ModuleType("antenv.axon_hooks")
    mod._hook = _hook
    mod.get_axon_ntff_profile_hook = lambda: mod._hook
    mod.set_axon_ntff_profile_hook = lambda h: setattr(mod, "_hook", h)
    sys.modules["antenv.axon_hooks"] = mod
    try:
        import antenv

        antenv.axon_hooks = mod
    except ImportError:
        pass


def _build():
    import concourse.bass as bass
    import concourse.tile as tile
    from concourse import bacc, mybir

    F32 = mybir.dt.float32
    BF16 = mybir.dt.bfloat16
    U32 = mybir.dt.uint32
    Exp = mybir.ActivationFunctionType.Exp
    mult = mybir.AluOpType.mult
    add = mybir.AluOpType.add
    sub = mybir.AluOpType.subtract
    bxor = mybir.AluOpType.bitwise_xor
    shr = mybir.AluOpType.logical_shift_right
    bypass = mybir.AluOpType.bypass

    nc = bacc.Bacc("TRN2", target_bir_lowering=False, debug=False, num_devices=N_CORES)

    xbT = nc.dram_tensor("xbT", (D, T), BF16, kind="ExternalInput").ap()
    wq_s = nc.dram_tensor("wq_s", (D, HPC * HD), BF16, kind="ExternalInput").ap()
    wk_s = nc.dram_tensor("wk_s", (D, HPC * HD), BF16, kind="ExternalInput").ap()
    wv_s = nc.dram_tensor("wv_s", (D, HPC * HD), BF16, kind="ExternalInput").ap()
    wo_s = nc.dram_tensor("wo_s", (HPC * HD, D), BF16, kind="ExternalInput").ap()
    ident_d = nc.dram_tensor("ident", (128, 128), BF16, kind="ExternalInput").ap()
    bd2_d = nc.dram_tensor("bd2", (128, 2), BF16, kind="ExternalInput").ap()
    wqc_d = nc.dram_tensor("wqc", (128, 1), F32, kind="ExternalInput").ap()
    wkc_d = nc.dram_tensor("wkc", (128, 1), F32, kind="ExternalInput").ap()
    outT = nc.dram_tensor("outT", (D, T), BF16, kind="ExternalOutput").ap()

    def dram_view(tl, shape):
        """raw row-major AP view over a DRAM tile's buffer"""
        ap = tl[:]
        strides = []
        s = 1
        for n in reversed(shape):
            strides.append([s, n])
            s *= n
        return bass.AP(tensor=ap.tensor, offset=ap.offset, ap=list(reversed(strides)))

    def bcast_row(row_ap, n=64):
        """partition-broadcast a [1, cols] AP to n partitions"""
        return bass.AP(tensor=row_ap.tensor, offset=row_ap.offset,
                       ap=[[0, n]] + list(row_ap.ap[1:]))

    with tile.TileContext(nc) as tc:
        from contextlib import ExitStack

        with ExitStack() as top:
            consts = top.enter_context(tc.tile_pool(name="consts", bufs=1))
            xtp = top.enter_context(tc.tile_pool(name="xT", bufs=1))
            wpool = top.enter_context(tc.tile_pool(name="w", bufs=1))
            qtp = top.enter_context(tc.tile_pool(name="qt", bufs=1))
            q2p = top.enter_context(tc.tile_pool(name="q2", bufs=2))
            msp = top.enter_context(tc.tile_pool(name="ms", bufs=1))
            hatp = top.enter_context(tc.tile_pool(name="hat", bufs=1))
            vpp = top.enter_context(tc.tile_pool(name="vp", bufs=1))
            pbp = top.enter_context(tc.tile_pool(name="pb", bufs=4))
            dntp = top.enter_context(tc.tile_pool(name="dnt", bufs=2))
            ohpp = top.enter_context(tc.tile_pool(name="ohp", bufs=1))
            rwp = top.enter_context(tc.tile_pool(name="rw", bufs=2))
            packp = top.enter_context(tc.tile_pool(name="pack", bufs=2))
            osbp = top.enter_context(tc.tile_pool(name="osb", bufs=3))
            psS = top.enter_context(tc.tile_pool(name="psS", bufs=2, space="PSUM"))
            psO = top.enter_context(tc.tile_pool(name="psO", bufs=1, space="PSUM"))
            psX = top.enter_context(tc.tile_pool(name="psX", bufs=2, space="PSUM"))
            drp = top.enter_context(tc.tile_pool(name="dr", bufs=1, space="DRAM"))

            # qhat/khat[h]: [128, T] bf16; head h=2p+i data in rows 64i:64i+64,
            # complement rows must be zero (contraction zero-padding)
            qhat = [hatp.tile([128, T], BF16, tag=f"qh{h}", name=f"qhat{h}")
                    for h in range(HPC)]
            khat = [hatp.tile([128, T], BF16, tag=f"kh{h}", name=f"khat{h}")
                    for h in range(HPC)]
            for h in range(HPC):
                i = h % 2
                rows = slice(64 * (1 - i), 64 * (1 - i) + 64)
                nc.gpsimd.memset(qhat[h][rows, :], 0.0)
                nc.gpsimd.memset(khat[h][rows, :], 0.0)
            # vp[pair]: [128 keys, kt, head-in-pair, 65] bf16; col 64 = 1.0
            vp = [vpp.tile([128, TT, 2, 65], BF16, tag=f"v{p}", name=f"vp{p}")
                  for p in range(NPAIR)]
            for p in range(NPAIR):
                nc.vector.memset(vp[p][:, :, :, 64:65], 1.0)

            # ---------------- input DMA (consumption order) -------------------
            wk_sb = [wpool.tile([128, 256], BF16, tag=f"wk{c}", name=f"wk{c}")
                     for c in range(CT)]
            wq_sb = [wpool.tile([128, 256], BF16, tag=f"wq{c}", name=f"wq{c}")
                     for c in range(CT)]
            wv_sb = [wpool.tile([128, 256], BF16, tag=f"wv{c}", name=f"wv{c}")
                     for c in range(CT)]
            xT = [xtp.tile([128, T], BF16, tag=f"xT{c}", name=f"xT{c}")
                  for c in range(CT)]
            for c in range(CT):
                nc.sync.dma_start(out=wk_sb[c][:], in_=wk_s[c * 128:(c + 1) * 128, :])
                nc.sync.dma_start(out=xT[c][:], in_=xbT[c * 128:(c + 1) * 128, :])
            for c in range(CT):
                nc.sync.dma_start(out=wq_sb[c][:], in_=wq_s[c * 128:(c + 1) * 128, :])
            for c in range(CT):
                nc.sync.dma_start(out=wv_sb[c][:], in_=wv_s[c * 128:(c + 1) * 128, :])
            wo_sb = []
            for cp in range(NPAIR):
                wt = wpool.tile([128, D], BF16, tag=f"wo{cp}", name=f"wo{cp}")
                nc.gpsimd.dma_start(out=wt[:], in_=wo_s[cp * 128:(cp + 1) * 128, :])
                wo_sb.append(wt)
            ident = consts.tile([128, 128], BF16, tag="ident")
            nc.gpsimd.dma_start(out=ident[:], in_=ident_d)
            bd2 = consts.tile([128, 2], BF16, tag="bd2")
            nc.gpsimd.dma_start(out=bd2[:], in_=bd2_d)
            wqc = consts.tile([128, 1], F32, tag="wqc")
            nc.gpsimd.dma_start(out=wqc[:], in_=wqc_d)
            wkc = consts.tile([128, 1], F32, tag="wkc")
            nc.gpsimd.dma_start(out=wkc[:], in_=wkc_d)

            qt = {}      # (proj, pair) -> [128, T] bf16 staging
            ms = {}      # (proj, pair) -> [2, T] bf16 sumsq rows
            vt_sb = {}   # pair -> [128, T] bf16 V^T staging
            rstdD = {}   # (proj, pair) -> DRAM [2, T] bf16
            msD = {}
            for proj in ("k", "q"):
                for p in range(NPAIR):
                    qt[(proj, p)] = qtp.tile([128, T], BF16, tag=f"qt{proj}{p}",
                                             name=f"qt{proj}{p}")
                    ms[(proj, p)] = msp.tile([2, T], BF16, tag=f"ms{proj}{p}",
                                             name=f"ms{proj}{p}")
                    msD[(proj, p)] = drp.tile([2, T], BF16, tag=f"msD{proj}{p}",
                                              name=f"msD{proj}{p}")
                    rstdD[(proj, p)] = drp.tile([2, T], BF16, tag=f"rsD{proj}{p}",
                                                name=f"rsD{proj}{p}")
            for p in range(NPAIR):
                vt_sb[p] = qtp.tile([128, T], BF16, tag=f"vt{p}", name=f"vt{p}")

            ohp = [ohpp.tile([128, T], BF16, tag=f"ohp{p}", name=f"ohp{p}")
                   for p in range(NPAIR)]
            ohr = [ohpp.tile([128, T], BF16, tag=f"ohr{p}", name=f"ohr{p}")
                   for p in range(NPAIR)]
            dnD = [drp.tile([HPC, 1024], BF16, tag=f"dnD{qh}", name=f"dnD{qh}")
                   for qh in range(QH)]
            rcD = [drp.tile([HPC, 1024], BF16, tag=f"rcD{qh}", name=f"rcD{qh}")
                   for qh in range(QH)]

            W_SB = {"k": wk_sb, "q": wq_sb, "v": wv_sb}

            # ---------------- building blocks ---------------------------------
            def proj_chunk(proj, pair, qh, qq):
                dest = vt_sb[pair] if proj == "v" else qt[(proj, pair)]
                pj = psX.tile([128, 512], F32, tag="x")
                off = qh * 1024 + qq * 512
                for ct in range(CT):
                    nc.tensor.matmul(
                        pj[:], W_SB[proj][ct][:, pair * 128:(pair + 1) * 128],
                        xT[ct][:, off:off + 512],
                        start=(ct == 0), stop=(ct == CT - 1))
                with nc.allow_low_precision(reason="bf16 staging"):
                    nc.vector.tensor_copy(out=dest[:, off:off + 512], in_=pj[:])

            def sumsq_chunk(proj, pair, qh):
                q2 = q2p.tile([128, 1024], BF16, tag="q2")
                sl = slice(qh * 1024, (qh + 1) * 1024)
                with nc.allow_low_precision(reason="bf16 sumsq"):
                    nc.vector.tensor_tensor(out=q2[:], in0=qt[(proj, pair)][:, sl],
                                            in1=qt[(proj, pair)][:, sl], op=mult)
                for qq in range(2):
                    ss = psX.tile([128, 512], F32, tag="x")
                    nc.tensor.matmul(ss[0:2, :], bd2[:], q2[:, qq * 512:(qq + 1) * 512],
                                     start=True, stop=True)
                    # ACT is idle pre-attention; Copy lives in every act table
                    with nc.allow_low_precision(reason="bf16 ms"):
                        nc.scalar.copy(
                            ms[(proj, pair)][:, qh * 1024 + qq * 512:
                                             qh * 1024 + (qq + 1) * 512],
                            ss[0:2, :])

            def rstd_unit(proj, pair):
                """rstdD = (ms/64+eps)^-1/2; runs on GPSIMD so the DVE queue
                never stalls on the DMA pack round-trip"""
                nc.sync.dma_start(out=msD[(proj, pair)][:], in_=ms[(proj, pair)][:])
                mp = packp.tile([128, 32], BF16, tag="pk16", name=f"mp{proj}{pair}")
                nc.sync.dma_start(out=mp[:], in_=dram_view(msD[(proj, pair)], [128, 32]))
                m = packp.tile([128, 32], F32, tag="pk32a", name=f"m{proj}{pair}")
                mh = packp.tile([128, 32], F32, tag="pk32b", name=f"mh{proj}{pair}")
                y = packp.tile([128, 32], F32, tag="pk32c", name=f"y{proj}{pair}")
                t1 = packp.tile([128, 32], F32, tag="pk32d", name=f"t1{proj}{pair}")
                t2 = packp.tile([128, 32], F32, tag="pk32e", name=f"t2{proj}{pair}")
                rs = packp.tile([128, 32], BF16, tag="pk16b", name=f"rs{proj}{pair}")
                ts = nc.vector.tensor_scalar
                tt = nc.vector.tensor_tensor
                with nc.allow_low_precision(reason="rstd chain"):
                    ts(out=m[:], in0=mp[:], scalar1=1.0 / HD, scalar2=EPS,
                       op0=mult, op1=add)
                    ts(out=mh[:], in0=m[:], scalar1=0.5, scalar2=0.0,
                       op0=mult, op1=bypass)
                    # y0 = bitcast(((~bits) >> 1) - 0x20A8C620)
                    ts(out=y[:].bitcast(U32), in0=m[:].bitcast(U32),
                       scalar1=0xFFFFFFFF, scalar2=1, op0=bxor, op1=shr)
                    ts(out=y[:].bitcast(U32), in0=y[:].bitcast(U32),
                       scalar1=0x20A8C620, scalar2=0, op0=sub, op1=bypass)
                    for _ in range(2):  # two sign-cancelling Newton steps
                        tt(out=t1[:], in0=mh[:], in1=y[:], op=mult)
                        tt(out=t2[:], in0=t1[:], in1=y[:], op=mult)
                        ts(out=t2[:], in0=t2[:], scalar1=1.5, scalar2=0.0,
                           op0=sub, op1=bypass)
                        tt(out=y[:], in0=t2[:], in1=y[:], op=mult)
                    nc.vector.tensor_copy(out=rs[:], in_=y[:])
                nc.sync.dma_start(out=dram_view(rstdD[(proj, pair)], [128, 32]),
                                  in_=rs[:])

            def norm_chunk(proj, pair, qh):
                """qhat/khat[2p+i][64i:64i+64, :] = qt * wcol * rstd (bf16)"""
                dest = qhat if proj == "q" else khat
                wcol = wqc if proj == "q" else wkc
                rw = rwp.tile([128, 1024], BF16, tag="rwn")
                sl = slice(qh * 1024, (qh + 1) * 1024)
                for i in range(2):
                    nc.sync.dma_start(
                        out=rw[64 * i:64 * i + 64, :],
                        in_=bcast_row(rstdD[(proj, pair)][i:i + 1, sl]))
                with nc.allow_low_precision(reason="bf16 qkhat"):
                    for i in range(2):
                        rows = slice(64 * i, 64 * i + 64)
                        nc.vector.scalar_tensor_tensor(
                            out=dest[pair * 2 + i][rows, sl],
                            in0=qt[(proj, pair)][rows, sl],
                            scalar=wcol[rows, :],
                            in1=rw[rows, :],
                            op0=mult, op1=mult)

            def vtrans_chunk(pair, tq):
                """vt_sb[pair] kt 4tq..4tq+3 -> vp[pair] bf16 [V|1] weights"""
                vb = psX.tile([128, 512], F32, tag="x")
                vbb = vb[:].bitcast(BF16)  # [128, 1024] bf16 view
                for b_ in range(4):
                    kt = tq * 4 + b_
                    nc.tensor.transpose(vbb[:, b_ * 128:(b_ + 1) * 128],
                                        vt_sb[pair][:, kt * 128:(kt + 1) * 128],
                                        ident[:])
                for b_ in range(4):
                    kt = tq * 4 + b_
                    with nc.allow_low_precision(reason="bf16 v"):
                        nc.vector.tensor_copy(
                            out=vp[pair][:, kt, :, 0:64],
                            in_=vbb[:, b_ * 128:(b_ + 1) * 128].rearrange(
                                "p (h d) -> p h d", h=2))

            # ---------------- pre-attention: pair-0 pipeline ------------------
            for qh in range(QH):
                for qq in range(2):
                    proj_chunk("k", 0, qh, qq)
            for qh in range(QH):
                sumsq_chunk("k", 0, qh)
            rstd_unit("k", 0)
            for qh in range(QH):
                for qq in range(2):
                    proj_chunk("q", 0, qh, qq)
            for qh in range(QH):
                sumsq_chunk("q", 0, qh)
            rstd_unit("q", 0)
            for qh in range(QH):
                norm_chunk("k", 0, qh)
            norm_chunk("q", 0, 0)
            for qh in range(QH):
                for qq in range(2):
                    proj_chunk("v", 0, qh, qq)
            norm_chunk("q", 0, 1)
            for tq in range(4):
                vtrans_chunk(0, tq)

            # pair-1 work, finely chunked, interleaved under heads 0-1 of the
            # first query half; fully drained before head 2 is emitted
            bg = []
            for proj in ("k", "q"):
                for qh in range(QH):
                    for qq in range(2):
                        bg.append(lambda proj=proj, qh=qh, qq=qq:
                                  proj_chunk(proj, 1, qh, qq))
                for qh in range(QH):
                    bg.append(lambda proj=proj, qh=qh: sumsq_chunk(proj, 1, qh))
                bg.append(lambda proj=proj: rstd_unit(proj, 1))
            for proj in ("k", "q"):
                for qh in range(QH):
                    bg.append(lambda proj=proj, qh=qh: norm_chunk(proj, 1, qh))
            for qh in range(QH):
                for qq in range(2):
                    bg.append(lambda qh=qh, qq=qq: proj_chunk("v", 1, qh, qq))
            for tq in range(4):
                bg.append(lambda tq=tq: vtrans_chunk(1, tq))

            def pop_bg(n=1):
                for _ in range(n):
                    if bg:
                        bg.pop(0)()

            # ---------------- attention + output projection -------------------
            for qh in range(QH):
                for h in range(HPC):
                    pair, i = h // 2, h % 2
                    o_ps = psO.tile([65, 1024], F32, tag="o")
                    for kt in range(TT):
                        s_ps = psS.tile([128, 1024], F32, tag="s")
                        for qq in range(2):
                            nc.tensor.matmul(
                                s_ps[:, qq * 512:(qq + 1) * 512],
                                khat[h][:, kt * 128:(kt + 1) * 128],
                                qhat[h][:, qh * 1024 + qq * 512:
                                        qh * 1024 + (qq + 1) * 512],
                                start=True, stop=True)
                        pb = pbp.tile([128, 1024], BF16, tag="pb")
                        nc.scalar.activation(pb[:], s_ps[:], Exp, scale=0.125)
                        for qq in range(2):
                            nc.tensor.matmul(
                                o_ps[:, qq * 512:(qq + 1) * 512],
                                vp[pair][:, kt, i, :],
                                pb[:, qq * 512:(qq + 1) * 512],
                                start=(kt == 0), stop=(kt == TT - 1))
                        if qh == 0 and h < 2 and kt % 2 == 1:
                            pop_bg(2)
                    # evict unnormalized O + denominator row
                    sl = slice(qh * 1024, (qh + 1) * 1024)
                    dnt = dntp.tile([65, 1024], BF16, tag="dnt")
                    with nc.allow_low_precision(reason="bf16 o"):
                        nc.vector.tensor_copy(out=dnt[:], in_=o_ps[:])
                    nc.gpsimd.dma_start(out=dnD[qh][h:h + 1, :], in_=dnt[64:65, :])
                    nc.gpsimd.dma_start(out=ohp[pair][64 * i:64 * i + 64, sl],
                                        in_=dnt[0:64, :])
                    if qh == 0 and h == 1:
                        pop_bg(len(bg))  # drain before head 2 needs pair-1 data
                # denominator reciprocals for this query half (all 4 heads)
                dnp16 = packp.tile([128, 32], BF16, tag="pk16", name=f"dn{qh}")
                nc.sync.dma_start(out=dnp16[:], in_=dram_view(dnD[qh], [128, 32]))
                dnf = packp.tile([128, 32], F32, tag="pk32a", name=f"dnf{qh}")
                rcf = packp.tile([128, 32], F32, tag="pk32b", name=f"rcf{qh}")
                rcs = packp.tile([128, 32], F32, tag="pk32c", name=f"rcs{qh}")
                rc16 = packp.tile([128, 32], BF16, tag="pk16b", name=f"rc16{qh}")
                with nc.allow_low_precision(reason="recip"):
                    nc.vector.tensor_copy(out=dnf[:], in_=dnp16[:])
                    nc.vector.reciprocal_approx_accurate(out=rcf[:], in_=dnf[:],
                                                         scratch=rcs[:])
                    nc.vector.tensor_copy(out=rc16[:], in_=rcf[:])
                nc.sync.dma_start(out=dram_view(rcD[qh], [128, 32]), in_=rc16[:])
                sl = slice(qh * 1024, (qh + 1) * 1024)
                for pair in range(NPAIR):
                    rw = rwp.tile([128, 1024], BF16, tag="rwo", name=f"rwo{qh}{pair}")
                    for i in range(2):
                        nc.sync.dma_start(
                            out=rw[64 * i:64 * i + 64, :],
                            in_=bcast_row(rcD[qh][2 * pair + i:2 * pair + i + 1, :]))
                    with nc.allow_low_precision(reason="bf16 ohat"):
                        nc.vector.tensor_tensor(out=ohr[pair][:, sl],
                                                in0=ohp[pair][:, sl],
                                                in1=rw[:], op=mult)
                # output projection for this token half
                for et in range(D // 128):
                    for qq in range(2):
                        ops = psX.tile([128, 512], F32, tag="x")
                        for cp in range(NPAIR):
                            nc.tensor.matmul(
                                ops[:], wo_sb[cp][:, et * 128:(et + 1) * 128],
                                ohr[cp][:, qh * 1024 + qq * 512:
                                        qh * 1024 + (qq + 1) * 512],
                                start=(cp == 0), stop=(cp == NPAIR - 1))
                        osb = osbp.tile([128, 512], BF16, tag="osb")
                        with nc.allow_low_precision(reason="bf16 out"):
                            if qq == 0:
                                nc.vector.tensor_copy(out=osb[:], in_=ops[:])
                            else:
                                nc.scalar.copy(osb[:], ops[:])
                        nc.gpsimd.dma_start(
                            out=outT[et * 128:(et + 1) * 128,
                                     qh * 1024 + qq * 512:qh * 1024 + (qq + 1) * 512],
                            in_=osb[:])

    nc.compile()
    return nc


def _get_compiled():
    global _COMPILED
    if _COMPILED is None:
        _COMPILED = _build()
    return _COMPILED


def kernel(x, wq, wk, wv, wo, q_norm_w, k_norm_w):
    import ml_dtypes
    from concourse.bass_utils import run_bass_kernel_spmd

    global LAST_EXEC_NS
    if os.environ.get("BASS_TRACE"):
        _install_ntff_shim()

    x = np.asarray(x, dtype=np.float32)
    wq = np.asarray(wq, dtype=np.float32)
    wk = np.asarray(wk, dtype=np.float32)
    wv = np.asarray(wv, dtype=np.float32)
    wo = np.asarray(wo, dtype=np.float32)
    q_norm_w = np.asarray(q_norm_w, dtype=np.float32)
    k_norm_w = np.asarray(k_norm_w, dtype=np.float32)

    nc = _get_compiled()
    bf = lambda a: a.astype(ml_dtypes.bfloat16)

    ident = np.eye(128, dtype=np.float32)
    bd2 = np.zeros((128, 2), np.float32)
    bd2[0:64, 0] = 1.0
    bd2[64:128, 1] = 1.0
    wqc = np.concatenate([q_norm_w, q_norm_w]).reshape(128, 1).astype(np.float32)
    wkc = np.concatenate([k_norm_w, k_norm_w]).reshape(128, 1).astype(np.float32)

    in_maps = []
    for c in range(N_CORES):
        b = c // 4
        hs = HPC * (c % 4)
        # head split in reference is strided: head h uses columns d*H + h
        perm = ((hs + np.arange(HPC))[:, None] + H * np.arange(HD)[None, :]).reshape(-1)
        in_maps.append({
            "xbT": bf(np.ascontiguousarray(x[b].T)),
            "wq_s": bf(np.ascontiguousarray(wq[:, perm])),
            "wk_s": bf(np.ascontiguousarray(wk[:, perm])),
            "wv_s": bf(np.ascontiguousarray(wv[:, perm])),
            "wo_s": bf(np.ascontiguousarray(wo[hs * HD:(hs + HPC) * HD, :])),
            "ident": bf(ident), "bd2": bf(bd2),
            "wqc": wqc, "wkc": wkc,
        })

    res = run_bass_kernel_spmd(nc, in_maps, core_ids=list(range(N_CORES)),
                               trace=bool(os.environ.get("BASS_TRACE")),
                               tmpdir=os.environ.get("BASS_TRACE_DIR"))
    LAST_EXEC_NS = res.exec_time_ns

    out = np.empty((B, T, D), dtype=np.float32)
    for b in range(B):
        acc = res.results[4 * b]["outT"].astype(np.float32)
        for c in range(4 * b + 1, 4 * b + 4):
            acc = acc + res.results[c]["outT"].astype(np.float32)
        out[b] = acc.T
    return out
